# revision 45
# baseline (speedup 1.0000x reference)
"""GPTQ int4 linear kernel for Trainium2, 8-way sharded over out_features (v2).

Computes y = x @ W_dq^T + bias where W_dq is group-dequantized from int4
nibbles packed two-per-int32 (only the low byte of each int32 is used).

Host-side: the int32 qweight (values 0..255) is repacked to its raw bytes
(a strided memcpy) so the device streams 2.88 MB/core instead of 11.5 MB.
Viewed as int16, each lane v holds two packed bytes = 4 weights:
    n0 = v & 15, n1 = (v >> 4) & 15, n2 = (v >> 8) & 15, n3 = v >> 12

Device algorithm (per core, shard of 1376 out-rows padded to 1408):
  - qb16 [1408, 1024] int16 is DMA-xbar-transposed in 8 column chunks into
    v_j [128 lanes, 1408 o] tiles (lane 128j+p = weights k=4(128j+p)..+3,
    so tile j covers groups 4j..4j+3 with group(p) = 4j + p//32).
  - fp16 bit-pattern decode (0x6400|m is fp16(1024+m) for 0<=m<1024):
      A = (v & 0x0F) | 0x6400 -> 1024 + n0      (DVE, one tensor_scalar)
      B = (v & 0xF0) | 0x6400 -> 1024 + 16*n1   (DVE)
      w = v >> 8 (arithmetic ok; high bits are masked off below)  (Pool)
      C = (w & 0x0F) | 0x6400 -> 1024 + n2      (DVE)
      D = (w & 0xF0) | 0x6400 -> 1024 + 16*n3   (DVE)
  - Moving operands are group-masked fp16 x: mq[p, (g',b)] = x̂[b, k(p)]
    for p//32 == g' else 0 (planes B/D carry x̂/16, exact in fp16), so a
    single [128,128]x[128,128] matmul per (plane, o-tile) accumulates
    per-group partials P[o, (g',b)] = sum_{k in g} n_k x̂_k + 1024*S_AC +
    64*S_BD into PSUM (4 plane-matmuls per slot).
  - The constant offsets and the -8 nibble centering are cancelled by 11
    small f32 correction matmuls: rows zp_g vs true-x group sums, rows
    -s_g vs u_g = 1032*S_AC + 72*S_BD (host-computed sums of the
    fp16-ROUNDED x, so cancellation is exact), plus a bias row.
  - Eviction: Pool multiplies PSUM partials by s[4j+g', o] (scales are
    fp16-rounded on host so the correction cancels exactly) and Pool
    accumulates the 4 group columns of each tile into y^T.
Host only reshapes/pads inputs (byte-granularity memcpy, no weight-volume
arithmetic) and assembles the output.
"""

import sys

for _p in ("/opt/trn_rl_repo",):
    if _p not in sys.path:
        sys.path.insert(0, _p)

import numpy as np

import concourse.bacc as bacc
import concourse.bass as bass
import concourse.mybir as mybir
from concourse import tile
from concourse.bass_utils import run_bass_kernel_spmd

# Problem constants (hardcoded per contract)
OUT_F = 11008
IN_F = 4096
GROUP = 128
N_GROUPS = IN_F // GROUP  # 32
BATCH = 32
N_CORES = 8
SHARD = OUT_F // N_CORES      # 1376
SHARD_P = 1408                # padded to a multiple of 128
T = SHARD_P // 128            # 11 o-tiles
NJ = 8                        # lane tiles (1024 int16 lanes / 128)
NZ = 2 * N_GROUPS + 1         # correction rows: zp, -s, bias

F32 = mybir.dt.float32
F16 = mybir.dt.float16
I16 = mybir.dt.int16


def build_nc(out_p=SHARD_P, repeat=1, debug_skip=(), w_eng="dve", ev_eng="pool",
             tp_head=2, wp_bufs=3, v_bufs=2 * NJ, ps_bufs=2, in_bufs=4,
             unroll=8, plane_bufs=2, mm_dt="f16"):
    """Build the single-core program (identical across cores, data differs).

    debug_skip: timing-only ablations, subset of {"dma" (transposes),
      "unpack", "mm", "evict", "adds", "corr"}. Results wrong when used.
    w_eng/ev_eng: engine for the >>8 pass / eviction multiply.
    tp_head: how many transposes to issue before the input DMAs.
    unroll: bodies per hardware-loop iteration. Inside tc.For_i the body is
      emitted once, so pool tiles map to fixed buffers across hw iterations;
      unrolling U bodies makes the pools rotate between them, overlapping
      body i's tail with body i+1's DMAs/transposes (software pipelining).
    """
    nc = bacc.Bacc("TRN2", target_bir_lowering=False, debug=False)
    MMDT = {"f16": F16, "bf16": mybir.dt.bfloat16}[mm_dt]

    # weights stored pre-transposed on host: qt[j, p, o] = packed-byte-pair
    # lane 128j+p of output row o (the DMA-xbar transpose costs ~3.7us per
    # [1408,128] tile on real HW, ~3x the plain-DMA rate, so the transpose
    # is done once on the host as a byte permutation instead)
    qt_d = nc.dram_tensor("qt", [128, NJ * out_p], I16, kind="ExternalInput")
    # compact movings: only the 32 live columns per (plane, j); the zero mask
    # blocks live in fixed SBUF buffers memset once before the loop
    mq_d = nc.dram_tensor("mq", [128, 4 * NJ * BATCH], MMDT, kind="ExternalInput")
    # aux32 blob: scj [128, 0:352] | z [0:65, 352:1760] | xs [0:65, 1760:1792]
    AUXW = NJ * T * 4 + T * 128 + BATCH
    aux_d = nc.dram_tensor("aux", [128, AUXW], F32, kind="ExternalInput")
    yt_d = nc.dram_tensor("yT", [128, T * BATCH], F32, kind="ExternalOutput")

    ev = {"pool": "gpsimd", "dve": "vector"}[ev_eng]
    weng = {"pool": "gpsimd", "dve": "vector"}[w_eng]

    with tile.TileContext(nc) as tc:
        with (
            tc.tile_pool(name="xc", bufs=in_bufs) as xc,
            tc.tile_pool(name="wp", bufs=wp_bufs) as wp,
            tc.tile_pool(name="acc", bufs=in_bufs) as accp,
            tc.tile_pool(name="ps", bufs=ps_bufs, space="PSUM") as psp,
        ):
            import contextlib

            n_mqf = min(in_bufs, max(2, unroll))
            mqfs = [
                xc.tile([128, 4, NJ, 128], MMDT, tag="mqf", name=f"mqf{s}",
                        bufs=n_mqf)
                for s in range(n_mqf)
            ]
            for mqf in mqfs:
                nc.vector.memset(mqf[:], 0.0)
            # load the ACT function table once, outside the loop
            actwarm = xc.tile([128, 1], F32, tag="actwarm", bufs=1)
            nc.vector.memset(actwarm[:], 0.0)
            nc.scalar.activation(
                actwarm[:], actwarm[:], mybir.ActivationFunctionType.Copy
            )

            def body(it):
                mqc = xc.tile([128, 4, NJ, BATCH], MMDT, tag="mq", name=f"mq{it}")
                mq = mqfs[it % n_mqf]
                aux = xc.tile([128, AUXW], F32, tag="aux", name=f"aux{it}")
                y = accp.tile([128, T, BATCH], F32, tag="y", name=f"y{it}")
                y4 = accp.tile([128, T, 4, BATCH], F32, tag="y4", name=f"y4{it}")
                scj = aux[:, : NJ * T * 4].rearrange(
                    "p (j t g) -> p j t g", j=NJ, t=T
                )
                z = aux[:NZ, NJ * T * 4 : NJ * T * 4 + T * 128].rearrange(
                    "r (t c) -> r t c", t=T
                )
                xs = aux[:NZ, NJ * T * 4 + T * 128 :]

                qt = qt_d[:].rearrange("p (j o) -> p j o", j=NJ)

                def issue_tp(j, v, eng):
                    if "dma" in debug_skip:
                        return
                    eng.dma_start(v[:], qt[:, j])

                vts = {}
                for j in range(NJ):
                    vts[j] = wp.tile([128, out_p], I16, tag="v",
                                     name=f"v{it}g{j}", bufs=v_bufs)

                # All input DMAs ride the sync (SP) HWDGE ring: ACT's in-order
                # queue is reserved for the PSUM->SBUF eviction copies, so the
                # next body's DMAs are never stuck behind this body's copies.
                for j in range(2):
                    issue_tp(j, vts[j], nc.sync)
                nc.sync.dma_start(
                    mqc[:], mq_d[:].rearrange("p (m j c) -> p m j c", m=4, j=NJ)
                )
                issue_tp(2, vts[2], nc.sync)
                issue_tp(3, vts[3], nc.sync)
                nc.sync.dma_start(aux[:], aux_d[:])
                for j in range(4, NJ):
                    issue_tp(j, vts[j], nc.sync)
                # expand compact movings into the masked block-diagonal form
                for g_ in range(4):
                    nc.vector.tensor_copy(
                        mq[32 * g_ : 32 * (g_ + 1), :, :,
                           32 * g_ : 32 * (g_ + 1)],
                        mqc[32 * g_ : 32 * (g_ + 1)],
                    )

                psc = psp.tile([128, 512], F32, tag="psc", name=f"psc{it}",
                               bufs=min(2, ps_bufs))

                def emit_corr():
                    # correction matmuls (f32): psc[:, t] = z[:, t].T @ xs.
                    # Emitted after the j=3 block so the in-order PE queue
                    # doesn't stall on the late z/xs DMAs.
                    if "corr" in debug_skip:
                        return
                    for t in range(T):
                        nc.tensor.matmul(
                            psc[:, 32 * t : 32 * (t + 1)],
                            z[:, t],
                            xs[:],
                            start=True,
                            stop=True,
                        )

                def emit_evict(j, psj, ssc):
                    # eviction: GPSIMD cannot touch PSUM on real HW, so the
                    # idle ACT engine copies the partials to SBUF f32, then
                    # Pool applies the scales. All APs stay fully contiguous
                    # ([p, t, g, b] throughout): strided 32-element runs cost
                    # ~2.5x on the real engines. y4 keeps the 4 group columns
                    # separate; they are folded once per body.
                    if "evict" in debug_skip or "mm" in debug_skip:
                        return
                    # DVE reads PSUM directly (ACT copies and Pool-PSUM are
                    # both slower paths on real HW)
                    mult_out = ssc if j > 0 else y4
                    nc.vector.tensor_tensor(
                        mult_out[:],
                        psj[:, :T].rearrange("p t (g b) -> p t g b", g=4),
                        scj[:, j].unsqueeze(3).broadcast_to([128, T, 4, BATCH]),
                        mybir.AluOpType.mult,
                    )
                    if "adds" in debug_skip or j == 0:
                        return
                    nc.gpsimd.tensor_tensor(
                        y4[:], y4[:], ssc[:], mybir.AluOpType.add
                    )

                for j in range(NJ):
                    v = vts[j]
                    A = wp.tile([128, out_p], I16, tag="A", bufs=plane_bufs)
                    B = wp.tile([128, out_p], I16, tag="B", bufs=plane_bufs)
                    C = wp.tile([128, out_p], I16, tag="C", bufs=plane_bufs)
                    D = wp.tile([128, out_p], I16, tag="D", bufs=plane_bufs)
                    w = wp.tile([128, out_p], I16, tag="w", bufs=plane_bufs)
                    if "unpack" not in debug_skip:
                        getattr(nc, weng).tensor_scalar(
                            w[:], v[:], 8, None,
                            mybir.AluOpType.logical_shift_right,
                        )
                        nc.vector.tensor_scalar(
                            A[:], v[:], 0x0F, 0x6400,
                            mybir.AluOpType.bitwise_and, mybir.AluOpType.bitwise_or,
                        )
                        nc.vector.tensor_scalar(
                            B[:], v[:], 0xF0, 0x6400,
                            mybir.AluOpType.bitwise_and, mybir.AluOpType.bitwise_or,
                        )
                        nc.vector.tensor_scalar(
                            C[:], w[:], 0x0F, 0x6400,
                            mybir.AluOpType.bitwise_and, mybir.AluOpType.bitwise_or,
                        )
                        nc.vector.tensor_scalar(
                            D[:], w[:], 0xF0, 0x6400,
                            mybir.AluOpType.bitwise_and, mybir.AluOpType.bitwise_or,
                        )
                    psj = psp.tile([128, 12, 128], F32, tag="psj")
                    if "mm" not in debug_skip:
                        # t-outer: a start=True marks the whole 2KB psum bank
                        # pending-zero, so each slot's 4-matmul accumulation
                        # group must fully close before the next slot in the
                        # same bank opens.
                        for t in range(T):
                            for pi, plane in enumerate((A, B, C, D)):
                                nc.tensor.matmul(
                                    psj[:, t],
                                    plane[:, 128 * t : 128 * (t + 1)].bitcast(MMDT),
                                    mq[:, pi, j],
                                    start=(pi == 0),
                                    stop=(pi == 3),
                                )
                    ssc = wp.tile([128, T, 4, BATCH], F32, tag="ssc")
                    emit_evict(j, psj, ssc)
                    if j == 3:
                        emit_corr()
                full = not (
                    {"mm", "evict", "adds", "corr"} & set(debug_skip)
                )
                if full:
                    pscc = wp.tile([128, T, BATCH], F32, tag="pscc",
                                   bufs=min(2, ps_bufs))
                    nc.scalar.activation(
                        pscc[:],
                        psc[:, : T * BATCH].rearrange("p (t b) -> p t b", t=T),
                        mybir.ActivationFunctionType.Copy,
                    )
                    # fold the 4 group columns + correction into y
                    u0 = wp.tile([128, T, BATCH], F32, tag="u0", bufs=2)
                    nc.gpsimd.tensor_tensor(
                        u0[:], y4[:, :, 0], y4[:, :, 1], mybir.AluOpType.add
                    )
                    nc.gpsimd.tensor_tensor(
                        u0[:], u0[:], y4[:, :, 2], mybir.AluOpType.add
                    )
                    nc.gpsimd.tensor_tensor(
                        u0[:], u0[:], y4[:, :, 3], mybir.AluOpType.add
                    )
                    nc.gpsimd.tensor_tensor(
                        y[:], pscc[:], u0[:], mybir.AluOpType.add
                    )
                    nc.gpsimd.dma_start(
                        yt_d[:].rearrange("p (t b) -> p t b", t=T), y[:]
                    )
                else:
                    # ablation fallback: y is (partially) unwritten
                    nc.gpsimd.dma_start(
                        yt_d[:].rearrange("p (t b) -> p t b", t=T),
                        aux[:, : T * BATCH].rearrange("p (t b) -> p t b", t=T),
                    )

            if repeat == 1:
                body(0)
            else:
                U = unroll
                while repeat % U:
                    U -= 1
                with tc.For_i(
                    0, repeat // U, 1,
                    hint_engines=(
                        mybir.EngineType.PE,
                        mybir.EngineType.DVE,
                        mybir.EngineType.SP,
                        mybir.EngineType.Activation,
                        mybir.EngineType.Pool,
                    ),
                ):
                    for it in range(U):
                        body(it)

    nc.compile()
    return nc


def prep_inputs(x, qweight_packed, scales, zero_points, bias, perm, out_p=SHARD_P,
                n_cores=N_CORES):
    """Host-side sharding/reshaping. Byte-granularity memcpy + small-tensor
    compute only (sums/scales over [*, 32] group tensors)."""
    x = np.asarray(x, np.float32)
    qweight_packed = np.ascontiguousarray(np.asarray(qweight_packed, np.int32))
    # scales are rounded to fp16 so the device-side correction cancels exactly
    s16 = np.asarray(scales, np.float32).astype(np.float16).astype(np.float32)
    zero_points = np.asarray(zero_points, np.float32)
    bias = np.asarray(bias, np.float32)
    perm = np.asarray(perm, np.int64)
    shard = qweight_packed.shape[0] // n_cores

    # raw packed bytes: low byte of each little-endian int32
    qb = np.ascontiguousarray(
        qweight_packed.view(np.uint8).reshape(OUT_F, IN_F // 2, 4)[:, :, 0]
    )
    qb16_full = qb.view(np.int16)  # [OUT_F, 1024]; lane l = weights 4l..4l+3

    x_perm = x[:, perm]                       # [B, IN_F]
    xh = x_perm.astype(np.float16)            # fp16-rounded x (moving operand)
    xk = np.ascontiguousarray(xh.T).reshape(IN_F // 4, 4, BATCH)  # [lane, m, b]

    # compact movings mq[p, m, j, b] (the device expands them into the
    # group-masked block-diagonal layout)
    mq = np.ascontiguousarray(
        xk.reshape(NJ, 128, 4, BATCH).transpose(1, 2, 0, 3)
    )  # [p, m, j, b]
    mq[:, 1] /= np.float16(16)
    mq[:, 3] /= np.float16(16)
    mq_sb = np.ascontiguousarray(mq).reshape(128, 4 * NJ * BATCH)

    # group sums of the fp16-rounded x, split by k mod 4 parity
    xr4 = xk.astype(np.float64)                           # [lane, m, b]
    g_of_lane = xr4.reshape(N_GROUPS, 32, 4, BATCH)       # [g, lane-in-g, m, b]
    S_AC = (g_of_lane[:, :, 0] + g_of_lane[:, :, 2]).sum(axis=1)  # [g, b]
    S_BD = (g_of_lane[:, :, 1] + g_of_lane[:, :, 3]).sum(axis=1)
    S_all = (
        x_perm.astype(np.float64).T.reshape(N_GROUPS, GROUP, BATCH).sum(axis=1)
    )                                                      # true-x sums [g, b]
    u = 1032.0 * S_AC + 72.0 * S_BD
    xs = np.concatenate(
        [S_all, u, np.ones((1, BATCH))], axis=0
    ).astype(np.float32)                                   # [65, B]

    in_maps = []
    AUXW = NJ * T * 4 + T * 128 + BATCH
    for c in range(n_cores):
        sl = slice(c * shard, (c + 1) * shard)
        qt = np.zeros((128, NJ, out_p), np.int16)
        # qt[p, j, o] = lane 128j+p of row o (pre-transposed weight layout)
        qt[:, :, :shard] = qb16_full[sl].T.reshape(NJ, 128, shard).transpose(1, 0, 2)
        s_pad = np.zeros((out_p, N_GROUPS), np.float32)
        s_pad[:shard] = s16[sl]
        z = np.zeros((NZ, out_p), np.float32)
        z[:N_GROUPS, :shard] = zero_points[sl].T
        z[N_GROUPS : 2 * N_GROUPS] = -s_pad.T
        z[2 * N_GROUPS, :shard] = bias[sl]
        scj = s_pad.reshape(T, 128, NJ, 4).transpose(1, 2, 0, 3).reshape(
            128, NJ * T * 4
        )
        aux = np.zeros((128, AUXW), np.float32)
        aux[:, : NJ * T * 4] = scj
        aux[:NZ, NJ * T * 4 : NJ * T * 4 + T * 128] = z
        aux[:NZ, NJ * T * 4 + T * 128 :] = xs
        in_maps.append(
            {
                "qt": np.ascontiguousarray(qt).reshape(128, NJ * out_p),
                "mq": mq_sb,
                "aux": aux,
            }
        )
    return in_maps


def assemble_output(results, out_p=SHARD_P, n_cores=N_CORES, shard=SHARD):
    cols = []
    for c in range(n_cores):
        yt = np.asarray(results[c]["yT"], np.float32)     # [128, T*B]
        yc = yt.reshape(128, T, BATCH).transpose(2, 1, 0).reshape(BATCH, out_p)
        cols.append(yc[:, :shard])
    return np.concatenate(cols, axis=1)


class _Runner:
    """Builds the program once and keeps one jitted sharded executable so
    repeated calls (and timing loops) reuse the same axon mesh executable."""

    def __init__(self, **build_kwargs):
        import jax
        from jax.sharding import Mesh, PartitionSpec, NamedSharding
        from jax.experimental.shard_map import shard_map
        from concourse import bass2jax

        self.jax = jax
        self.nc = build_nc(**build_kwargs)
        bass2jax.install_neuronx_cc_hook()
        nc = self.nc
        partition_name = (
            nc.partition_id_tensor.name if nc.partition_id_tensor else None
        )
        in_names, out_names, out_avals, zero_outs = [], [], [], []
        for alloc in nc.m.functions[0].allocations:
            if not isinstance(alloc, mybir.MemoryLocationSet):
                continue
            name = alloc.memorylocations[0].name
            if alloc.kind == "ExternalInput":
                if name != partition_name:
                    in_names.append(name)
            elif alloc.kind == "ExternalOutput":
                out_names.append(name)
                shape = tuple(alloc.tensor_shape)
                dtype = mybir.dt.np(alloc.dtype)
                out_avals.append(jax.core.ShapedArray(shape, dtype))
                zero_outs.append(np.zeros(shape, dtype))
        self.in_names, self.out_names = in_names, out_names
        self.out_avals, self.zero_outs = out_avals, zero_outs
        n_params, n_outs = len(in_names), len(out_avals)
        all_names = tuple(in_names + out_names)
        if partition_name is not None:
            all_names = all_names + (partition_name,)

        def _body(*args):
            operands = list(args)
            if partition_name is not None:
                operands.append(bass2jax.partition_id_tensor())
            outs = bass2jax._bass_exec_p.bind(
                *operands,
                out_avals=tuple(out_avals),
                in_names=all_names,
                out_names=tuple(out_names),
                lowering_input_output_aliases=(),
                sim_require_finite=True,
                sim_require_nnan=True,
                nc=nc,
            )
            return tuple(outs)

        devices = jax.devices()[:N_CORES]
        self.mesh = Mesh(np.asarray(devices), ("core",))
        in_specs = (PartitionSpec("core"),) * (n_params + n_outs)
        out_specs = (PartitionSpec("core"),) * n_outs
        self.sharded = jax.jit(
            shard_map(
                _body, mesh=self.mesh, in_specs=in_specs, out_specs=out_specs,
                check_rep=False,
            ),
            donate_argnums=tuple(range(n_params, n_params + n_outs)),
            keep_unused=True,
        )
        self.sharding = NamedSharding(self.mesh, PartitionSpec("core"))

    def put_inputs(self, in_maps):
        jax = self.jax
        arrs = [
            jax.device_put(
                np.concatenate(
                    [np.asarray(in_maps[c][n]) for c in range(N_CORES)], axis=0
                ),
                self.sharding,
            )
            for n in self.in_names
        ]
        for a in arrs:
            a.block_until_ready()
        return arrs

    def execute(self, dev_inputs):
        jax = self.jax
        zs = [
            jax.device_put(
                np.zeros((N_CORES * z.shape[0], *z.shape[1:]), z.dtype), self.sharding
            )
            for z in self.zero_outs
        ]
        for z in zs:
            z.block_until_ready()
        outs = self.sharded(*dev_inputs, *zs)
        jax.block_until_ready(outs)
        return outs

    def run(self, in_maps):
        outs = self.execute(self.put_inputs(in_maps))
        res = []
        for c in range(N_CORES):
            d = {}
            for i, name in enumerate(self.out_names):
                d[name] = np.asarray(outs[i]).reshape(
                    N_CORES, *self.out_avals[i].shape
                )[c]
            res.append(d)
        return res


_RUNNER_CACHE = {}


def get_runner(**build_kwargs):
    key = tuple(sorted(build_kwargs.items()))
    if key not in _RUNNER_CACHE:
        _RUNNER_CACHE[key] = _Runner(**build_kwargs)
    return _RUNNER_CACHE[key]


def kernel(x, qweight_packed, scales, zero_points, bias, perm):
    runner = get_runner()
    in_maps = prep_inputs(x, qweight_packed, scales, zero_points, bias, perm)
    return assemble_output(runner.run(in_maps))


# revision 47
# speedup vs baseline: 1.0236x; 1.0236x over previous
"""GPTQ int4 linear kernel for Trainium2, 8-way sharded over out_features (v2).

Computes y = x @ W_dq^T + bias where W_dq is group-dequantized from int4
nibbles packed two-per-int32 (only the low byte of each int32 is used).

Host-side: the int32 qweight (values 0..255) is repacked to its raw bytes
(a strided memcpy) so the device streams 2.88 MB/core instead of 11.5 MB.
Viewed as int16, each lane v holds two packed bytes = 4 weights:
    n0 = v & 15, n1 = (v >> 4) & 15, n2 = (v >> 8) & 15, n3 = v >> 12

Device algorithm (per core, shard of 1376 out-rows padded to 1408):
  - Weights arrive pre-transposed from the host (qt[p, j, o] = int16 lane
    128j+p of row o; the DMA-xbar transpose measures ~3x slower than plain
    DMA on real HW, so the transpose is a host-side byte permutation) and
    stream in as 8 plain [128, 1408] tiles per iteration (2.88 MB/core,
    1/4 of the naive int32 volume).
  - fp16 bit-pattern decode on DVE (0x6400|m is fp16(1024+m), m<1024):
      A = (v & 0x0F) | 0x6400 -> 1024 + n0
      B = (v & 0xF0) | 0x6400 -> 1024 + 16*n1
      w = v >> 8 (sign smear is masked off below)
      C = (w & 0x0F) | 0x6400 -> 1024 + n2
      D = (w & 0xF0) | 0x6400 -> 1024 + 16*n3
  - Moving operands are group-masked fp16 x (expanded on device from a
    compact DMA into fixed zeroed buffers): mq[p, (g',b)] = x̂[b, k(p)] for
    p//32 == g' else 0 (planes B/D carry x̂/16, exact in fp16), so each
    [128,128]x[128,128] matmul accumulates per-group partials
    P[o, (g',b)] = sum_{k in g} n_k x̂_k + 1024*S_AC + 64*S_BD into PSUM
    (4 plane-matmuls per (j, o-tile) slot, t-outer because start=True
    marks the whole 2KB psum bank pending-zero).
  - The constant offsets and the -8 nibble centering are cancelled by 11
    small f32 correction matmuls: rows zp_g vs true-x group sums, rows
    -s_g vs u_g = 1032*S_AC + 72*S_BD (host-computed sums of the
    fp16-ROUNDED x, so cancellation is exact), plus a bias row.
  - Eviction: only DVE/ACT may read PSUM (walrus rejects GPSIMD-PSUM), and
    f32 tensor-tensor ops cost ~2x the nominal rate on every engine, so
    the scale multiply alternates per tile between DVE-direct-from-PSUM
    and ACT-copy + Pool-multiply; Pool accumulates whole [128, T, 4, B]
    tiles into a 4-wide accumulator (fully contiguous APs; strided
    32-element runs measure ~2.5x slower) and folds the group columns
    once per iteration.
  - The whole body is unrolled 8x inside the hardware loop: tc.For_i
    emits the body once so pool tiles are fixed buffers across hw
    iterations; unrolling rotates them, hiding the per-loop DMA
    semaphore-reset drain (~15 us) and cross-body dependencies.
Host only reshapes/pads inputs (byte-granularity memcpy, no weight-volume
arithmetic) and assembles the output.
"""

import sys

for _p in ("/opt/trn_rl_repo",):
    if _p not in sys.path:
        sys.path.insert(0, _p)

import numpy as np

import concourse.bacc as bacc
import concourse.bass as bass
import concourse.mybir as mybir
from concourse import tile
from concourse.bass_utils import run_bass_kernel_spmd

# Problem constants (hardcoded per contract)
OUT_F = 11008
IN_F = 4096
GROUP = 128
N_GROUPS = IN_F // GROUP  # 32
BATCH = 32
N_CORES = 8
SHARD = OUT_F // N_CORES      # 1376
SHARD_P = 1408                # padded to a multiple of 128
T = SHARD_P // 128            # 11 o-tiles
NJ = 8                        # lane tiles (1024 int16 lanes / 128)
NZ = 2 * N_GROUPS + 1         # correction rows: zp, -s, bias

F32 = mybir.dt.float32
F16 = mybir.dt.float16
I16 = mybir.dt.int16


def build_nc(out_p=SHARD_P, repeat=1, debug_skip=(), w_eng="dve", ev_eng="pool",
             tp_head=2, wp_bufs=3, v_bufs=2 * NJ, ps_bufs=2, in_bufs=4,
             unroll=8, plane_bufs=2, mm_dt="f16"):
    """Build the single-core program (identical across cores, data differs).

    debug_skip: timing-only ablations, subset of {"dma" (transposes),
      "unpack", "mm", "evict", "adds", "corr"}. Results wrong when used.
    w_eng/ev_eng: engine for the >>8 pass / eviction multiply.
    tp_head: how many transposes to issue before the input DMAs.
    unroll: bodies per hardware-loop iteration. Inside tc.For_i the body is
      emitted once, so pool tiles map to fixed buffers across hw iterations;
      unrolling U bodies makes the pools rotate between them, overlapping
      body i's tail with body i+1's DMAs/transposes (software pipelining).
    """
    nc = bacc.Bacc("TRN2", target_bir_lowering=False, debug=False)
    MMDT = {"f16": F16, "bf16": mybir.dt.bfloat16}[mm_dt]

    # weights stored pre-transposed on host: qt[j, p, o] = packed-byte-pair
    # lane 128j+p of output row o (the DMA-xbar transpose costs ~3.7us per
    # [1408,128] tile on real HW, ~3x the plain-DMA rate, so the transpose
    # is done once on the host as a byte permutation instead)
    qt_d = nc.dram_tensor("qt", [128, NJ * out_p], I16, kind="ExternalInput")
    # compact movings: only the 32 live columns per (plane, j); the zero mask
    # blocks live in fixed SBUF buffers memset once before the loop
    mq_d = nc.dram_tensor("mq", [128, 4 * NJ * BATCH], MMDT, kind="ExternalInput")
    # aux32 blob: scj [128, 0:352] | z [0:65, 352:1760] | xs [0:65, 1760:1792]
    AUXW = NJ * T * 4 + T * 128 + BATCH
    aux_d = nc.dram_tensor("aux", [128, AUXW], F32, kind="ExternalInput")
    yt_d = nc.dram_tensor("yT", [128, T * BATCH], F32, kind="ExternalOutput")

    ev = {"pool": "gpsimd", "dve": "vector"}[ev_eng]
    weng = {"pool": "gpsimd", "dve": "vector"}[w_eng]

    with tile.TileContext(nc) as tc:
        with (
            tc.tile_pool(name="xc", bufs=in_bufs) as xc,
            tc.tile_pool(name="wp", bufs=wp_bufs) as wp,
            tc.tile_pool(name="acc", bufs=in_bufs) as accp,
            tc.tile_pool(name="ps", bufs=ps_bufs, space="PSUM") as psp,
        ):
            import contextlib

            n_mqf = min(in_bufs, max(2, unroll))
            mqfs = [
                xc.tile([128, 4, NJ, 128], MMDT, tag="mqf", name=f"mqf{s}",
                        bufs=n_mqf)
                for s in range(n_mqf)
            ]
            for mqf in mqfs:
                nc.vector.memset(mqf[:], 0.0)
            # load the ACT function table once, outside the loop
            actwarm = xc.tile([128, 1], F32, tag="actwarm", bufs=1)
            nc.vector.memset(actwarm[:], 0.0)
            nc.scalar.activation(
                actwarm[:], actwarm[:], mybir.ActivationFunctionType.Copy
            )

            def body(it):
                mqc = xc.tile([128, 4, NJ, BATCH], MMDT, tag="mq", name=f"mq{it}")
                mq = mqfs[it % n_mqf]
                aux = xc.tile([128, AUXW], F32, tag="aux", name=f"aux{it}")
                y = accp.tile([128, T, BATCH], F32, tag="y", name=f"y{it}")
                y4 = accp.tile([128, T, 4, BATCH], F32, tag="y4", name=f"y4{it}")
                scj = aux[:, : NJ * T * 4].rearrange(
                    "p (j t g) -> p j t g", j=NJ, t=T
                )
                z = aux[:NZ, NJ * T * 4 : NJ * T * 4 + T * 128].rearrange(
                    "r (t c) -> r t c", t=T
                )
                xs = aux[:NZ, NJ * T * 4 + T * 128 :]

                qt = qt_d[:].rearrange("p (j o) -> p j o", j=NJ)

                def issue_tp(j, v, eng):
                    if "dma" in debug_skip:
                        return
                    eng.dma_start(v[:], qt[:, j])

                vts = {}
                for j in range(NJ):
                    vts[j] = wp.tile([128, out_p], I16, tag="v",
                                     name=f"v{it}g{j}", bufs=v_bufs)

                # All input DMAs ride the sync (SP) HWDGE ring: ACT's in-order
                # queue is reserved for the PSUM->SBUF eviction copies, so the
                # next body's DMAs are never stuck behind this body's copies.
                for j in range(2):
                    issue_tp(j, vts[j], nc.sync)
                nc.sync.dma_start(
                    mqc[:], mq_d[:].rearrange("p (m j c) -> p m j c", m=4, j=NJ)
                )
                issue_tp(2, vts[2], nc.sync)
                issue_tp(3, vts[3], nc.sync)
                nc.sync.dma_start(aux[:], aux_d[:])
                for j in range(4, NJ):
                    issue_tp(j, vts[j], nc.sync)
                # expand compact movings into the masked block-diagonal form
                for g_ in range(4):
                    nc.vector.tensor_copy(
                        mq[32 * g_ : 32 * (g_ + 1), :, :,
                           32 * g_ : 32 * (g_ + 1)],
                        mqc[32 * g_ : 32 * (g_ + 1)],
                    )

                psc = psp.tile([128, 512], F32, tag="psc", name=f"psc{it}",
                               bufs=min(2, ps_bufs))

                def emit_corr():
                    # correction matmuls (f32): psc[:, t] = z[:, t].T @ xs.
                    # Emitted after the j=3 block so the in-order PE queue
                    # doesn't stall on the late z/xs DMAs.
                    if "corr" in debug_skip:
                        return
                    for t in range(T):
                        nc.tensor.matmul(
                            psc[:, 32 * t : 32 * (t + 1)],
                            z[:, t],
                            xs[:],
                            start=True,
                            stop=True,
                        )

                def emit_evict(j, psj, ssc):
                    # eviction: GPSIMD cannot touch PSUM on real HW, so the
                    # idle ACT engine copies the partials to SBUF f32, then
                    # Pool applies the scales. All APs stay fully contiguous
                    # ([p, t, g, b] throughout): strided 32-element runs cost
                    # ~2.5x on the real engines. y4 keeps the 4 group columns
                    # separate; they are folded once per body.
                    if "evict" in debug_skip or "mm" in debug_skip:
                        return
                    # the scale multiply must read PSUM, which only DVE and
                    # ACT may touch. DVE is the busiest engine (it also runs
                    # the 5 unpack passes), so alternate: odd j go direct on
                    # DVE, even j go ACT-copy -> Pool-mult.
                    mult_out = ssc if j > 0 else y4
                    psview = psj[:, :T].rearrange("p t (g b) -> p t g b", g=4)
                    scview = scj[:, j].unsqueeze(3).broadcast_to(
                        [128, T, 4, BATCH]
                    )
                    if j % 2:
                        nc.vector.tensor_tensor(
                            mult_out[:], psview, scview, mybir.AluOpType.mult
                        )
                    else:
                        psjc = wp.tile([128, T, 4, BATCH], F32, tag="psjc",
                                       bufs=plane_bufs)
                        nc.scalar.activation(
                            psjc[:], psview, mybir.ActivationFunctionType.Copy
                        )
                        nc.gpsimd.tensor_tensor(
                            mult_out[:], psjc[:], scview, mybir.AluOpType.mult
                        )
                    if "adds" in debug_skip or j == 0:
                        return
                    nc.gpsimd.tensor_tensor(
                        y4[:], y4[:], ssc[:], mybir.AluOpType.add
                    )

                for j in range(NJ):
                    v = vts[j]
                    A = wp.tile([128, out_p], I16, tag="A", bufs=plane_bufs)
                    B = wp.tile([128, out_p], I16, tag="B", bufs=plane_bufs)
                    C = wp.tile([128, out_p], I16, tag="C", bufs=plane_bufs)
                    D = wp.tile([128, out_p], I16, tag="D", bufs=plane_bufs)
                    w = wp.tile([128, out_p], I16, tag="w", bufs=plane_bufs)
                    if "unpack" not in debug_skip:
                        getattr(nc, weng).tensor_scalar(
                            w[:], v[:], 8, None,
                            mybir.AluOpType.logical_shift_right,
                        )
                        nc.vector.tensor_scalar(
                            A[:], v[:], 0x0F, 0x6400,
                            mybir.AluOpType.bitwise_and, mybir.AluOpType.bitwise_or,
                        )
                        nc.vector.tensor_scalar(
                            B[:], v[:], 0xF0, 0x6400,
                            mybir.AluOpType.bitwise_and, mybir.AluOpType.bitwise_or,
                        )
                        nc.vector.tensor_scalar(
                            C[:], w[:], 0x0F, 0x6400,
                            mybir.AluOpType.bitwise_and, mybir.AluOpType.bitwise_or,
                        )
                        nc.vector.tensor_scalar(
                            D[:], w[:], 0xF0, 0x6400,
                            mybir.AluOpType.bitwise_and, mybir.AluOpType.bitwise_or,
                        )
                    psj = psp.tile([128, 12, 128], F32, tag="psj")
                    if "mm" not in debug_skip:
                        # t-outer: a start=True marks the whole 2KB psum bank
                        # pending-zero, so each slot's 4-matmul accumulation
                        # group must fully close before the next slot in the
                        # same bank opens.
                        for t in range(T):
                            for pi, plane in enumerate((A, B, C, D)):
                                nc.tensor.matmul(
                                    psj[:, t],
                                    plane[:, 128 * t : 128 * (t + 1)].bitcast(MMDT),
                                    mq[:, pi, j],
                                    start=(pi == 0),
                                    stop=(pi == 3),
                                )
                    ssc = wp.tile([128, T, 4, BATCH], F32, tag="ssc")
                    emit_evict(j, psj, ssc)
                    if j == 3:
                        emit_corr()
                full = not (
                    {"mm", "evict", "adds", "corr"} & set(debug_skip)
                )
                if full:
                    pscc = wp.tile([128, T, BATCH], F32, tag="pscc",
                                   bufs=min(2, ps_bufs))
                    nc.scalar.activation(
                        pscc[:],
                        psc[:, : T * BATCH].rearrange("p (t b) -> p t b", t=T),
                        mybir.ActivationFunctionType.Copy,
                    )
                    # fold the 4 group columns + correction into y
                    u0 = wp.tile([128, T, BATCH], F32, tag="u0", bufs=2)
                    nc.gpsimd.tensor_tensor(
                        u0[:], y4[:, :, 0], y4[:, :, 1], mybir.AluOpType.add
                    )
                    nc.gpsimd.tensor_tensor(
                        u0[:], u0[:], y4[:, :, 2], mybir.AluOpType.add
                    )
                    nc.gpsimd.tensor_tensor(
                        u0[:], u0[:], y4[:, :, 3], mybir.AluOpType.add
                    )
                    nc.gpsimd.tensor_tensor(
                        y[:], pscc[:], u0[:], mybir.AluOpType.add
                    )
                    nc.gpsimd.dma_start(
                        yt_d[:].rearrange("p (t b) -> p t b", t=T), y[:]
                    )
                else:
                    # ablation fallback: y is (partially) unwritten
                    nc.gpsimd.dma_start(
                        yt_d[:].rearrange("p (t b) -> p t b", t=T),
                        aux[:, : T * BATCH].rearrange("p (t b) -> p t b", t=T),
                    )

            if repeat == 1:
                body(0)
            else:
                U = unroll
                while repeat % U:
                    U -= 1
                with tc.For_i(
                    0, repeat // U, 1,
                    hint_engines=(
                        mybir.EngineType.PE,
                        mybir.EngineType.DVE,
                        mybir.EngineType.SP,
                        mybir.EngineType.Activation,
                        mybir.EngineType.Pool,
                    ),
                ):
                    for it in range(U):
                        body(it)

    nc.compile()
    return nc


def prep_inputs(x, qweight_packed, scales, zero_points, bias, perm, out_p=SHARD_P,
                n_cores=N_CORES):
    """Host-side sharding/reshaping. Byte-granularity memcpy + small-tensor
    compute only (sums/scales over [*, 32] group tensors)."""
    x = np.asarray(x, np.float32)
    qweight_packed = np.ascontiguousarray(np.asarray(qweight_packed, np.int32))
    # scales are rounded to fp16 so the device-side correction cancels exactly
    s16 = np.asarray(scales, np.float32).astype(np.float16).astype(np.float32)
    zero_points = np.asarray(zero_points, np.float32)
    bias = np.asarray(bias, np.float32)
    perm = np.asarray(perm, np.int64)
    shard = qweight_packed.shape[0] // n_cores

    # raw packed bytes: low byte of each little-endian int32
    qb = np.ascontiguousarray(
        qweight_packed.view(np.uint8).reshape(OUT_F, IN_F // 2, 4)[:, :, 0]
    )
    qb16_full = qb.view(np.int16)  # [OUT_F, 1024]; lane l = weights 4l..4l+3

    x_perm = x[:, perm]                       # [B, IN_F]
    xh = x_perm.astype(np.float16)            # fp16-rounded x (moving operand)
    xk = np.ascontiguousarray(xh.T).reshape(IN_F // 4, 4, BATCH)  # [lane, m, b]

    # compact movings mq[p, m, j, b] (the device expands them into the
    # group-masked block-diagonal layout)
    mq = np.ascontiguousarray(
        xk.reshape(NJ, 128, 4, BATCH).transpose(1, 2, 0, 3)
    )  # [p, m, j, b]
    mq[:, 1] /= np.float16(16)
    mq[:, 3] /= np.float16(16)
    mq_sb = np.ascontiguousarray(mq).reshape(128, 4 * NJ * BATCH)

    # group sums of the fp16-rounded x, split by k mod 4 parity
    xr4 = xk.astype(np.float64)                           # [lane, m, b]
    g_of_lane = xr4.reshape(N_GROUPS, 32, 4, BATCH)       # [g, lane-in-g, m, b]
    S_AC = (g_of_lane[:, :, 0] + g_of_lane[:, :, 2]).sum(axis=1)  # [g, b]
    S_BD = (g_of_lane[:, :, 1] + g_of_lane[:, :, 3]).sum(axis=1)
    S_all = (
        x_perm.astype(np.float64).T.reshape(N_GROUPS, GROUP, BATCH).sum(axis=1)
    )                                                      # true-x sums [g, b]
    u = 1032.0 * S_AC + 72.0 * S_BD
    xs = np.concatenate(
        [S_all, u, np.ones((1, BATCH))], axis=0
    ).astype(np.float32)                                   # [65, B]

    in_maps = []
    AUXW = NJ * T * 4 + T * 128 + BATCH
    for c in range(n_cores):
        sl = slice(c * shard, (c + 1) * shard)
        qt = np.zeros((128, NJ, out_p), np.int16)
        # qt[p, j, o] = lane 128j+p of row o (pre-transposed weight layout)
        qt[:, :, :shard] = qb16_full[sl].T.reshape(NJ, 128, shard).transpose(1, 0, 2)
        s_pad = np.zeros((out_p, N_GROUPS), np.float32)
        s_pad[:shard] = s16[sl]
        z = np.zeros((NZ, out_p), np.float32)
        z[:N_GROUPS, :shard] = zero_points[sl].T
        z[N_GROUPS : 2 * N_GROUPS] = -s_pad.T
        z[2 * N_GROUPS, :shard] = bias[sl]
        scj = s_pad.reshape(T, 128, NJ, 4).transpose(1, 2, 0, 3).reshape(
            128, NJ * T * 4
        )
        aux = np.zeros((128, AUXW), np.float32)
        aux[:, : NJ * T * 4] = scj
        aux[:NZ, NJ * T * 4 : NJ * T * 4 + T * 128] = z
        aux[:NZ, NJ * T * 4 + T * 128 :] = xs
        in_maps.append(
            {
                "qt": np.ascontiguousarray(qt).reshape(128, NJ * out_p),
                "mq": mq_sb,
                "aux": aux,
            }
        )
    return in_maps


def assemble_output(results, out_p=SHARD_P, n_cores=N_CORES, shard=SHARD):
    cols = []
    for c in range(n_cores):
        yt = np.asarray(results[c]["yT"], np.float32)     # [128, T*B]
        yc = yt.reshape(128, T, BATCH).transpose(2, 1, 0).reshape(BATCH, out_p)
        cols.append(yc[:, :shard])
    return np.concatenate(cols, axis=1)


class _Runner:
    """Builds the program once and keeps one jitted sharded executable so
    repeated calls (and timing loops) reuse the same axon mesh executable."""

    def __init__(self, **build_kwargs):
        import jax
        from jax.sharding import Mesh, PartitionSpec, NamedSharding
        from jax.experimental.shard_map import shard_map
        from concourse import bass2jax

        self.jax = jax
        self.nc = build_nc(**build_kwargs)
        bass2jax.install_neuronx_cc_hook()
        nc = self.nc
        partition_name = (
            nc.partition_id_tensor.name if nc.partition_id_tensor else None
        )
        in_names, out_names, out_avals, zero_outs = [], [], [], []
        for alloc in nc.m.functions[0].allocations:
            if not isinstance(alloc, mybir.MemoryLocationSet):
                continue
            name = alloc.memorylocations[0].name
            if alloc.kind == "ExternalInput":
                if name != partition_name:
                    in_names.append(name)
            elif alloc.kind == "ExternalOutput":
                out_names.append(name)
                shape = tuple(alloc.tensor_shape)
                dtype = mybir.dt.np(alloc.dtype)
                out_avals.append(jax.core.ShapedArray(shape, dtype))
                zero_outs.append(np.zeros(shape, dtype))
        self.in_names, self.out_names = in_names, out_names
        self.out_avals, self.zero_outs = out_avals, zero_outs
        n_params, n_outs = len(in_names), len(out_avals)
        all_names = tuple(in_names + out_names)
        if partition_name is not None:
            all_names = all_names + (partition_name,)

        def _body(*args):
            operands = list(args)
            if partition_name is not None:
                operands.append(bass2jax.partition_id_tensor())
            outs = bass2jax._bass_exec_p.bind(
                *operands,
                out_avals=tuple(out_avals),
                in_names=all_names,
                out_names=tuple(out_names),
                lowering_input_output_aliases=(),
                sim_require_finite=True,
                sim_require_nnan=True,
                nc=nc,
            )
            return tuple(outs)

        devices = jax.devices()[:N_CORES]
        self.mesh = Mesh(np.asarray(devices), ("core",))
        in_specs = (PartitionSpec("core"),) * (n_params + n_outs)
        out_specs = (PartitionSpec("core"),) * n_outs
        self.sharded = jax.jit(
            shard_map(
                _body, mesh=self.mesh, in_specs=in_specs, out_specs=out_specs,
                check_rep=False,
            ),
            donate_argnums=tuple(range(n_params, n_params + n_outs)),
            keep_unused=True,
        )
        self.sharding = NamedSharding(self.mesh, PartitionSpec("core"))

    def put_inputs(self, in_maps):
        jax = self.jax
        arrs = [
            jax.device_put(
                np.concatenate(
                    [np.asarray(in_maps[c][n]) for c in range(N_CORES)], axis=0
                ),
                self.sharding,
            )
            for n in self.in_names
        ]
        for a in arrs:
            a.block_until_ready()
        return arrs

    def execute(self, dev_inputs):
        jax = self.jax
        zs = [
            jax.device_put(
                np.zeros((N_CORES * z.shape[0], *z.shape[1:]), z.dtype), self.sharding
            )
            for z in self.zero_outs
        ]
        for z in zs:
            z.block_until_ready()
        outs = self.sharded(*dev_inputs, *zs)
        jax.block_until_ready(outs)
        return outs

    def run(self, in_maps):
        outs = self.execute(self.put_inputs(in_maps))
        res = []
        for c in range(N_CORES):
            d = {}
            for i, name in enumerate(self.out_names):
                d[name] = np.asarray(outs[i]).reshape(
                    N_CORES, *self.out_avals[i].shape
                )[c]
            res.append(d)
        return res


_RUNNER_CACHE = {}


def get_runner(**build_kwargs):
    key = tuple(sorted(build_kwargs.items()))
    if key not in _RUNNER_CACHE:
        _RUNNER_CACHE[key] = _Runner(**build_kwargs)
    return _RUNNER_CACHE[key]


def kernel(x, qweight_packed, scales, zero_points, bias, perm):
    runner = get_runner()
    in_maps = prep_inputs(x, qweight_packed, scales, zero_points, bias, perm)
    return assemble_output(runner.run(in_maps))
